# revision 1
# baseline (speedup 1.0000x reference)
"""GNN message-passing aggregator on 8 Trainium2 NeuronCores — gatherless design.

  h = relu(relu(z @ U1 + c1) @ U2 + c2)
  z = segment_sum(relu(relu(y[src] @ W1 + b1) @ W2 + b2), dst)

Strategy (v2, replaces the dma_gather design: Pool desc-gen was 1.66ms):
  * MLP(y[src]) == MLP(y)[src]: compute m = MLP(y) once per node (phase 1),
    keep it in SBUF as bf16 [128 x NJ x 64] organized by host-chosen J-groups
    of 128 srcs (y is fed pre-permuted so no on-chip shuffle is needed).
  * Edges are sharded by dst across cores; each core's 6250 dsts form 49
    windows of 128 (greedy degree-balanced).  48 windows go through a dense
    (J, I, s<=8) bucket grid (host balances J-groups with a 32-choice greedy
    so bucket overflow is tiny); window 49 + overflow go through a small
    dma_gather sidecar (pair-indexed bf16 m table in DRAM).
  * Grid pipeline per pass (16 windows): PE expand (one matmul per J: one-hot
    Q [src x slot] x m_J -> G slots in PSUM) -> DVE copy to bf16 staging ->
    DMA to a DRAM bounce buffer in (I, J, s, f) order -> DMA back to SBUF in
    (part=(J%16,s), I, J//16, f) order -> PE scatter (lhsT=G column, rhs =
    one-hot P of dst labels) accumulating z^T [64 x 128] per window in PSUM.
  * Phase 3 consumes z^T directly (no transposes): ub1 matmul + relu + flip
    to [node x 64] via the second matmul, relu, DMA out.
  * All one-hots are built by DVE is_equal from host-streamed label arrays
    (Q labels replicated across partitions, int8 by default).
"""

import os

import numpy as np

# ---------------------------------------------------------------- constants
N_NODES = 50000
D = 64
NC = 8
NPC = N_NODES // NC          # 6250 dsts per core
NW = 49                      # dst windows per core
NWG = 48                     # windows handled by the grid
PASSES = 3
IPP = 16                     # windows per pass
NJ = 400                     # src J-groups (128 srcs each)
SCAP = 8                     # bucket capacity (J, I)
MPOS = NJ * 128              # padded node positions (51200)
NPAIR = MPOS // 2            # m_dram pair rows
COLS_I = NJ * SCAP // 128    # scatter columns per window (25)
GSLOTS = NJ * IPP * SCAP     # grid slots per pass (51200)
ICH = 4                      # windows per dma2 chunk
OP_COLS = 8                  # sidecar gather columns per op

_COMPILED = {}


def _bf16():
    import ml_dtypes
    return ml_dtypes.bfloat16


# ------------------------------------------------------------ host schedule
def _host_schedule(src, dst):
    """Per-core: I windows (degree-balanced), J groups (overflow-balanced),
    bucket grid labels, sidecar columns. Returns (sched, per_core)."""
    bf16 = _bf16()
    rng = np.random.default_rng(12345)
    src = src.astype(np.int64)
    dst = dst.astype(np.int64)

    per_core_raw = []
    scols_max = np.zeros(NW, np.int64)

    for c in range(NC):
        lo = c * NPC
        sel = (dst >= lo) & (dst < lo + NPC)
        s_e = src[sel]
        d_e = dst[sel] - lo

        # --- I windows: greedy degree balance over 6250 dsts -> 49x128
        deg = np.bincount(d_e, minlength=NPC)
        order = np.argsort(-deg, kind="stable")
        wload = np.zeros(NW, np.int64)
        wcount = np.zeros(NW, np.int64)
        asg_I = np.zeros(NPC, np.int64)
        lab_I = np.zeros(NPC, np.int64)
        for n in order:
            avail = np.flatnonzero(wcount < 128)
            w = avail[np.argmin(wload[avail])]
            asg_I[n] = w
            lab_I[n] = wcount[w]
            wcount[w] += 1
            wload[w] += deg[n]
        I_e = asg_I[d_e]

        # --- J groups: 32-choice greedy bucket balancing over all 50000 srcs
        sdeg = np.bincount(s_e, minlength=N_NODES)
        sorder = np.argsort(-sdeg, kind="stable")
        es = np.argsort(s_e, kind="stable")
        ss = s_e[es]
        ii = I_e[es]
        starts = np.searchsorted(ss, np.arange(N_NODES))
        ends = np.searchsorted(ss, np.arange(N_NODES), side="right")
        cnt = np.zeros((NJ, NWG), np.int32)
        jcount = np.zeros(NJ, np.int32)
        asg_J = np.zeros(N_NODES, np.int64)
        lab_J = np.zeros(N_NODES, np.int64)
        for n in sorder:
            a, b = starts[n], ends[n]
            Is = ii[a:b]
            Is = Is[Is < NWG]
            cands = rng.integers(0, NJ, 32)
            cands = cands[jcount[cands] < 128]
            if len(cands) == 0:
                cands = np.flatnonzero(jcount < 128)[:32]
            if len(Is):
                sub = cnt[cands][:, Is]
                ov = np.maximum(sub + 1 - SCAP, 0).sum(1)
                j = cands[np.argmin(ov)]
            else:
                j = cands[np.argmin(jcount[cands])]
            asg_J[n] = j
            lab_J[n] = jcount[j]
            jcount[j] += 1
            if len(Is):
                np.add.at(cnt, (j, Is), 1)

        # --- bucket fill: first SCAP edges -> grid, rest + w48 -> sidecar
        J_e = asg_J[s_e]
        qlab = np.full((PASSES, NJ, SCAP, IPP), -1, np.int64)
        plab = np.full((PASSES, IPP, COLS_I, 128), -1, np.int64)
        fill = np.zeros((NJ, NWG), np.int64)
        side = [[] for _ in range(NW)]     # (pos, dstlab) per window
        pos_e = asg_J[s_e] * 128 + lab_J[s_e]      # J*128+lab
        dl_e = lab_I[d_e]
        for k in range(len(s_e)):
            I = I_e[k]
            if I < NWG:
                J = J_e[k]
                f = fill[J, I]
                if f < SCAP:
                    fill[J, I] = f + 1
                    p, Il = I // IPP, I % IPP
                    qlab[p, J, f, Il] = lab_J[s_e[k]]
                    # scatter col: q = (J%16)*8+s, c = J//16
                    plab[p, Il, J // 16, (J % 16) * SCAP + f] = dl_e[k]
                else:
                    side[I].append((pos_e[k], dl_e[k]))
            else:
                side[I].append((pos_e[k], dl_e[k]))

        scols = np.array([(len(side[I]) + 127) // 128 for I in range(NW)])
        scols_max = np.maximum(scols_max, scols)
        per_core_raw.append((qlab, plab, side, asg_J, lab_J, asg_I, lab_I))

    scols_max = np.maximum(scols_max, 1)   # >=1 col per window for shape unif.
    stot = int(scols_max.sum())
    sched = {"scols": scols_max.astype(int).tolist(), "stot": stot}

    import ml_dtypes
    f8 = ml_dtypes.float8_e4m3fn
    per_core = []
    for c in range(NC):
        qlab, plab, side, asg_J, lab_J, asg_I, lab_I = per_core_raw[c]
        # Q one-hot stream: [128 x PASSES*NJ*128] dense fp8 (0/1 exact)
        q_flat = qlab.reshape(PASSES, NJ, SCAP * IPP).reshape(-1)
        qhot = (q_flat[None, :] == np.arange(128, dtype=np.int64)[:, None])
        qlab_rep = np.ascontiguousarray(qhot.astype(f8))
        # P one-hot stream: [128 q x cols*128] fp8; phot[q, col*128+n]
        pl = plab.reshape(PASSES * IPP * COLS_I, 128)      # [col, q]
        ph = (pl.T[:, :, None] == np.arange(128, dtype=np.int64)[None, None, :])
        plab_2d = np.ascontiguousarray(
            ph.reshape(128, PASSES * IPP * COLS_I * 128).astype(f8))

        # sidecar: pad each window to scols_max[c] columns
        sc_idx = np.zeros((stot * 128,), np.int64)
        sc_lab = np.full((2, stot * 128), -1, np.int64)   # even/odd labels
        base = 0
        for I in range(NW):
            lst = side[I]
            for k, (pos, dlab) in enumerate(lst):
                J, lab = pos // 128, pos % 128
                sc_idx[base * 128 + k] = lab * (NJ // 2) + J // 2
                sc_lab[J % 2, base * 128 + k] = dlab
            base += int(sched["scols"][I])
        # wrap idx into per-op [16, ni/16] layout replicated to 128 partitions
        blocks = []
        posn = 0
        while posn < stot:
            k = min(OP_COLS, stot - posn)
            op = sc_idx[posn * 128:(posn + k) * 128].astype(np.int16)
            blk = op.reshape(k * 128 // 16, 16).T
            blocks.append(np.tile(blk, (8, 1)))
            posn += k
        scidx = np.ascontiguousarray(np.concatenate(blocks, axis=1))
        sclab = np.ascontiguousarray(
            sc_lab.reshape(2, stot, 128).transpose(2, 1, 0).reshape(128, stot * 2)
            .astype(bf16))
        # ^ [128 part x (col, parity)] : col-major pairs (even, odd)

        # permuted y^T (bf16) and output perm
        yT = np.zeros((D + 1, MPOS), np.float32)
        perm_m = np.full(MPOS, -1, np.int64)
        perm_m[asg_J * 128 + lab_J] = np.arange(N_NODES)
        valid = perm_m >= 0
        per_core.append({"perm_m": perm_m, "valid": valid})
        yTv = per_core[-1]

        perm_h = np.full(NW * 128, -1, np.int64)
        perm_h[asg_I * 128 + lab_I] = np.arange(NPC) + c * NPC

        yTv.update({
            "qlab": qlab_rep, "plab": plab_2d,
            "scidx": scidx, "sclab": sclab, "perm_h": perm_h,
        })
    return sched, per_core, per_core_raw


# ------------------------------------------------------------- bass program
def _build_program(sched):
    import concourse.bacc as bacc
    import concourse.mybir as mybir
    import concourse.tile as tile
    from concourse.tile import add_dep_helper

    f32 = mybir.dt.float32
    bf = mybir.dt.bfloat16
    i16 = mybir.dt.int16
    i8 = mybir.dt.int8
    Relu = mybir.ActivationFunctionType.Relu
    Copy = mybir.ActivationFunctionType.Copy

    scols = sched["scols"]
    stot = sched["stot"]
    f8 = mybir.dt.float8e4
    CHUNK = 512
    NCH = MPOS // CHUNK                    # phase-1 chunks (100)
    QCH = 6144                             # Q chunk: 3 slabs of 16J (3x2048)
    NQCH = PASSES * NJ * 128 // QCH
    assert NQCH * QCH == PASSES * NJ * 128 and QCH % 2048 == 0

    nc = bacc.Bacc()
    yT_in = nc.dram_tensor("yT", [D + 1, MPOS], bf, kind="ExternalInput")
    wb1_in = nc.dram_tensor("wb1", [D + 1, D], bf, kind="ExternalInput")
    wb2_in = nc.dram_tensor("wb2", [D + 1, D], bf, kind="ExternalInput")
    ub1_in = nc.dram_tensor("ub1", [D + 1, D], bf, kind="ExternalInput")
    ub2_in = nc.dram_tensor("ub2", [D + 1, D], bf, kind="ExternalInput")
    qlab_in = nc.dram_tensor("qlab", [128, PASSES * NJ * 128], f8,
                             kind="ExternalInput")
    plab_in = nc.dram_tensor("plab", [128, PASSES * IPP * COLS_I * 128], f8,
                             kind="ExternalInput")
    iota_in = nc.dram_tensor("iota128", [128, 128], bf, kind="ExternalInput")
    scidx_in = nc.dram_tensor("scidx", [128, stot * 8], i16, kind="ExternalInput")
    sclab_in = nc.dram_tensor("sclab", [128, stot * 2], bf, kind="ExternalInput")
    m_dram = nc.dram_tensor("m_pairs", [NPAIR, 128], bf, kind="Internal")
    gdram = [nc.dram_tensor(f"gbounce{p}", [16, SCAP, IPP, COLS_I, D], bf,
                            kind="Internal")
             for p in range(PASSES)]
    h_out = nc.dram_tensor("h_out", [NW * 128, D], f32, kind="ExternalOutput")

    with tile.TileContext(nc) as tc:
        with tc.tile_pool(name="const", bufs=1) as cpool, \
             tc.tile_pool(name="mtab", bufs=1) as mpool, \
             tc.tile_pool(name="scg", bufs=1) as scgp:
            wb1 = cpool.tile([D + 1, D], bf, tag="wb1")
            wb2 = cpool.tile([D + 1, D], bf, tag="wb2")
            ub1 = cpool.tile([D + 1, D], bf, tag="ub1")
            ub2 = cpool.tile([D + 1, D], bf, tag="ub2")
            iota = cpool.tile([128, 128], bf, tag="iota")
            sclab_t = cpool.tile([128, stot * 2], bf, tag="sclab")
            scidx_t = cpool.tile([128, stot * 8], i16, tag="scidx")
            nc.sync.dma_start(out=wb1[:], in_=wb1_in[:])
            nc.sync.dma_start(out=wb2[:], in_=wb2_in[:])
            nc.sync.dma_start(out=ub1[:], in_=ub1_in[:])
            nc.sync.dma_start(out=ub2[:], in_=ub2_in[:])
            nc.sync.dma_start(out=iota[:], in_=iota_in[:])
            nc.sync.dma_start(out=sclab_t[:], in_=sclab_in[:])
            nc.sync.dma_start(out=scidx_t[:], in_=scidx_in[:])

            m_sb = mpool.tile([128, NJ, D], bf, tag="m_sb")

            # ---------------- phase 1: m = MLP1(y) -> m_sb (bf16) ------------
            YCH = 2048
            with tc.tile_pool(name="p1y", bufs=3) as p1y, \
                 tc.tile_pool(name="p1h", bufs=1) as p1h, \
                 tc.tile_pool(name="p1ps", bufs=2, space="PSUM") as p1ps, \
                 tc.tile_pool(name="p1ps2", bufs=2, space="PSUM") as p1ps2:
                h1a = p1h.tile([D + 1, CHUNK], bf, tag="h1a")
                h1b = p1h.tile([D + 1, CHUNK], bf, tag="h1b")
                nc.gpsimd.memset(h1a[D:D + 1, :], 1.0)
                nc.gpsimd.memset(h1b[D:D + 1, :], 1.0)
                h1bufs = [h1a, h1b]
                for yt in range(MPOS // YCH):
                    ytile = p1y.tile([D + 1, YCH], bf, tag="ytile")
                    nc.scalar.dma_start(out=ytile[:],
                                        in_=yT_in[:, yt * YCH:(yt + 1) * YCH])
                    for sub in range(YCH // CHUNK):
                        ch = yt * (YCH // CHUNK) + sub
                        ps = p1ps.tile([D, CHUNK], f32, tag="ps1")
                        nc.tensor.matmul(
                            out=ps[:], lhsT=wb1[:],
                            rhs=ytile[:, sub * CHUNK:(sub + 1) * CHUNK],
                            start=True, stop=True)
                        h1c = h1bufs[ch % 2]
                        if ch % 2 == 0:
                            nc.scalar.activation(out=h1c[:D, :], in_=ps[:],
                                                 func=Relu)
                        else:
                            nc.vector.tensor_scalar_max(out=h1c[:D, :],
                                                        in0=ps[:], scalar1=0.0)
                        ps2 = p1ps2.tile([128, 4 * D], f32, tag="ps2")
                        for k in range(4):
                            nc.tensor.matmul(out=ps2[:, k * D:(k + 1) * D],
                                             lhsT=h1c[:, k * 128:(k + 1) * 128],
                                             rhs=wb2[:], start=True, stop=True)
                        nc.vector.tensor_scalar_max(
                            out=m_sb[:, ch * 4:(ch + 1) * 4, :],
                            in0=ps2[:].rearrange("p (t d) -> p t d", d=D),
                            scalar1=0.0)

            # m table dump (contiguous): pair row r = lab*(NJ/2) + J//2
            wr_m = nc.sync.dma_start(
                out=m_dram[:].rearrange("(p r) e -> p (r e)", p=128),
                in_=m_sb[:].rearrange("p j f -> p (j f)"))
            jm = nc.sync.nop(nofuse=True)
            add_dep_helper(jm.ins, wr_m.ins, sync=True, reason="m pairs ready")

            # sidecar gathers (Pool) — issue early, they overlap the grid
            g_tiles = {}
            with tc.tile_pool(name="sgat", bufs=1) as sgp:
                posn = 0
                oi = 0
                while posn < stot:
                    k = min(OP_COLS, stot - posn)
                    g = sgp.tile([128, k, 128], bf, tag=f"g{oi}")
                    ni = k * 128
                    gi = nc.gpsimd.dma_gather(
                        out_ap=g[:], in_ap=m_dram[:],
                        idxs_ap=scidx_t[:, posn * 8:posn * 8 + k * 8],
                        num_idxs=ni, num_idxs_reg=ni, elem_size=128)
                    add_dep_helper(gi.ins, jm.ins, sync=True,
                                   reason="gather after m ready")
                    for kk in range(k):
                        g_tiles[posn + kk] = (g, kk)
                    posn += k
                    oi += 1

                # ---------------- phase 2 + 3 ------------------------------
                scol_off = np.concatenate([[0], np.cumsum(scols)]).astype(int)
                with tc.tile_pool(name="qstr", bufs=3) as qstr, \
                     tc.tile_pool(name="poh", bufs=2) as poh, \
                     tc.tile_pool(name="stg", bufs=3) as stgp, \
                     tc.tile_pool(name="rbuf", bufs=4) as rbp, \
                     tc.tile_pool(name="pse", bufs=2, space="PSUM") as psep, \
                     tc.tile_pool(name="zt", bufs=2, space="PSUM") as ztp, \
                     tc.tile_pool(name="p3a", bufs=1, space="PSUM") as p3a, \
                     tc.tile_pool(name="p3b", bufs=1, space="PSUM") as p3b, \
                     tc.tile_pool(name="p3s", bufs=2) as p3s, \
                     tc.tile_pool(name="p3g", bufs=2) as p3g, \
                     tc.tile_pool(name="p3h", bufs=2) as p3h:

                    # qlab chunks: QCH slots each, issued just-in-time
                    qstream = {}

                    def ensure_qchunk(qc):
                        if qc not in qstream and qc < NQCH:
                            qt = qstr.tile([128, QCH], f8, tag="qs")
                            nc.sync.dma_start(
                                out=qt[:],
                                in_=qlab_in[:, qc * QCH:(qc + 1) * QCH])
                            qstream[qc] = qt
                        return qstream.get(qc)

                    def get_q(slot0, width):
                        """fp8 one-hot Q slice [128 x width] from the stream."""
                        qc, off = slot0 // QCH, slot0 % QCH
                        qt = ensure_qchunk(qc)
                        ensure_qchunk(qc + 1)
                        return qt[:, off:off + width]

                    dma1s = [[] for _ in range(PASSES)]
                    dma2s = [[] for _ in range(PASSES)]
                    zt_tiles = {}
                    h_grp = []

                    def expand_pass(p):
                        for slab in range(NJ // 16):         # 16 J per slab
                            J0 = slab * 16
                            s0 = (p * NJ + J0) * 128
                            q16 = get_q(s0, 16 * 128)
                            pse = psep.tile([128, 16 * D], f32, tag="pse")
                            for j in range(16):
                                nc.tensor.matmul(
                                    out=pse[:, j * D:(j + 1) * D],
                                    lhsT=q16[:, j * 128:(j + 1) * 128],
                                    rhs=m_sb[:, J0 + j, :],
                                    start=True, stop=True)
                            stg = stgp.tile([128, 16 * D], bf, tag="stg")
                            if slab % 3 == 2:
                                nc.scalar.copy(out=stg[:], in_=pse[:])
                            else:
                                nc.vector.tensor_copy(out=stg[:], in_=pse[:])
                            eng1 = nc.sync if slab % 2 == 0 else nc.scalar
                            d1 = eng1.dma_start(
                                out=gdram[p][:, :, :, slab, :]
                                    .rearrange("jj ss i f -> (ss i) jj f"),
                                in_=stg[:].rearrange("p (j f) -> p j f", f=D))
                            dma1s[p].append(d1)

                    def scatter_pass(p):
                        # dma2 chunks (ICH windows each), then scatter+ph3
                        for cidx in range(IPP // ICH):
                            rb = rbp.tile([128, ICH, COLS_I, D], bf, tag="rb")
                            d2 = nc.sync.dma_start(
                                out=rb[:].rearrange("q i c f -> q i (c f)"),
                                in_=gdram[p][:, :, cidx * ICH:(cidx + 1) * ICH]
                                    .rearrange("jj ss i c f -> (jj ss) i (c f)"))
                            for d1 in dma1s[p]:
                                add_dep_helper(d2.ins, d1.ins, sync=True,
                                               reason="bounce RAW")
                            dma2s[p].append((rb, d2))
                        zt4 = None
                        for Il in range(IPP):
                            I = p * IPP + Il
                            rb, _ = dma2s[p][Il // ICH]
                            ilc = Il % ICH
                            if Il % 4 == 0:
                                zt4 = ztp.tile([D, 4, 128], f32, tag="zt")
                            zt = zt4[:, Il % 4, :]
                            zt_tiles[I] = zt
                            # batched P one-hots for this window (25 cols)
                            pc0 = (p * IPP + Il) * COLS_I * 128
                            poh_t = poh.tile([128, COLS_I * 128], f8, tag="poh")
                            nc.gpsimd.dma_start(
                                out=poh_t[:],
                                in_=plab_in[:, pc0:pc0 + COLS_I * 128])
                            nsc = scols[I]
                            total = COLS_I + 2 * nsc
                            ci = 0
                            for cc in range(COLS_I):
                                nc.tensor.matmul(
                                    out=zt[:],
                                    lhsT=rb[:, ilc, cc, :],
                                    rhs=poh_t[:, cc * 128:(cc + 1) * 128],
                                    start=(ci == 0), stop=(ci == total - 1),
                                    skip_group_check=True)
                                ci += 1
                            ci = _sidecar(I, zt, ci, total)
                            if Il % 4 == 3:
                                phase3(p * IPP + Il - 3, 4)

                    def _sidecar(I, zt, ci, total):
                        for k in range(scols[I]):
                            col = int(scol_off[I]) + k
                            g, kk = g_tiles[col]
                            pv = poh.tile([128, 2, 128], bf, tag="pscol")
                            nc.vector.tensor_tensor(
                                out=pv[:],
                                in0=sclab_t[:, col * 2:col * 2 + 2, None]
                                    .to_broadcast([128, 2, 128]),
                                in1=iota[:, None, :].to_broadcast([128, 2, 128]),
                                op=mybir.AluOpType.is_equal)
                            for par in range(2):
                                nc.tensor.matmul(
                                    out=zt[:],
                                    lhsT=g[:, kk, par * D:(par + 1) * D],
                                    rhs=pv[:, par, :],
                                    start=(ci == 0), stop=(ci == total - 1),
                                    skip_group_check=True)
                                ci += 1
                        return ci

                    def phase3(I0, gw):
                        zt1 = p3s.tile([D + 1, 4 * 128], bf, tag="zt1")
                        nc.gpsimd.memset(zt1[D:D + 1, :gw * 128], 1.0)
                        for g in range(gw):
                            nc.scalar.copy(
                                out=zt1[:D, g * 128:(g + 1) * 128],
                                in_=zt_tiles[I0 + g])
                        psA = p3a.tile([D, 4 * 128], f32, tag="psA")
                        nc.tensor.matmul(out=psA[:, :gw * 128], lhsT=ub1[:],
                                         rhs=zt1[:, :gw * 128],
                                         start=True, stop=True)
                        g1 = p3g.tile([D + 1, 4 * 128], bf, tag="g1")
                        nc.scalar.activation(out=g1[:D, :gw * 128],
                                             in_=psA[:, :gw * 128], func=Relu)
                        nc.gpsimd.memset(g1[D:D + 1, :gw * 128], 1.0)
                        psB = p3b.tile([128, 4 * D], f32, tag="psB")
                        h_sb = p3h.tile([128, 4 * D], f32, tag="h_sb")
                        for g in range(gw):
                            nc.tensor.matmul(out=psB[:, g * D:(g + 1) * D],
                                             lhsT=g1[:, g * 128:(g + 1) * 128],
                                             rhs=ub2[:], start=True, stop=True)
                        nc.vector.tensor_scalar_max(
                            out=h_sb[:, :gw * D], in0=psB[:, :gw * D],
                            scalar1=0.0)
                        nc.sync.dma_start(
                            out=h_out[I0 * 128:(I0 + gw) * 128, :]
                                .rearrange("(t p) d -> p t d", p=128),
                            in_=h_sb[:, :gw * D].rearrange("p (t d) -> p t d",
                                                           d=D))

                    # schedule: all expands, then scatters (PE queue in-order)
                    expand_pass(0)
                    expand_pass(1)
                    expand_pass(2)
                    scatter_pass(0)
                    scatter_pass(1)
                    scatter_pass(2)
                    # window 48: sidecar only
                    I = NWG
                    zt48 = ztp.tile([D, 4, 128], f32, tag="zt")
                    zt = zt48[:, 0, :]
                    zt_tiles[I] = zt
                    nsc = scols[I]
                    ci = _sidecar(I, zt, 0, 2 * nsc)
                    phase3(NWG, 1)

    nc.compile()
    return nc


# ------------------------------------------------------------------- kernel
def kernel(**inputs):
    from concourse.bass_utils import run_bass_kernel_spmd

    bf16 = _bf16()
    y = np.asarray(inputs["y"], np.float32)
    src = np.asarray(inputs["src"])
    dst = np.asarray(inputs["dst"])
    Ws = {k: np.asarray(inputs[k], np.float32)
          for k in ("W1", "b1", "W2", "b2", "U1", "c1", "U2", "c2")}

    sched, per_core, _raw = _host_schedule(src, dst)
    key = (sched["stot"], tuple(sched["scols"]))
    if key not in _COMPILED:
        _COMPILED[key] = _build_program(sched)
    nc = _COMPILED[key]

    wb1 = np.concatenate([Ws["W1"], Ws["b1"][None, :]], 0).astype(bf16)
    wb2 = np.concatenate([Ws["W2"], Ws["b2"][None, :]], 0).astype(bf16)
    ub1 = np.concatenate([Ws["U1"], Ws["c1"][None, :]], 0).astype(bf16)
    ub2 = np.concatenate([Ws["U2"], Ws["c2"][None, :]], 0).astype(bf16)
    iota128 = np.tile(np.arange(128, dtype=np.float32), (128, 1)).astype(bf16)

    in_maps = []
    for c in range(NC):
        pc = per_core[c]
        yT = np.zeros((D + 1, MPOS), np.float32)
        valid = pc["valid"]
        yT[:D, valid] = y[pc["perm_m"][valid]].T
        yT[D, :] = 1.0
        in_maps.append({
            "yT": yT.astype(bf16), "wb1": wb1, "wb2": wb2, "ub1": ub1,
            "ub2": ub2, "qlab": pc["qlab"], "plab": pc["plab"],
            "iota128": iota128, "scidx": pc["scidx"],
            "sclab": pc["sclab"],
        })

    res = run_bass_kernel_spmd(nc, in_maps, core_ids=list(range(NC)),
                               trace=bool(int(os.environ.get("KERNEL_TRACE", "0"))))
    kernel.last_results = res
    kernel.last_exec_time_ns = res.exec_time_ns

    h_full = np.zeros((N_NODES, D), np.float32)
    for c in range(NC):
        out = res.results[c]["h_out"]
        perm_h = per_core[c]["perm_h"]
        valid = perm_h >= 0
        h_full[perm_h[valid]] = out[valid]
    return h_full

